# revision 64
# baseline (speedup 1.0000x reference)
"""MiniMax Lightning Attention on 8 Trainium2 NeuronCores.

Sharding: sequence-parallel. Core c handles batch c//4, token chunk
(c%4)*1024..+1024 (4 blocks of 256). The per-block decay-state recurrence
crosses chunk boundaries; each core computes its local per-chunk decay-
weighted KV summary E, an AllGather shares the 8 summaries, and each core
reconstructs its chunk-start state as a decay-weighted sum.

Fused single-residency design: all big activations (x, V, q, ys, gate)
stay SBUF-resident in bf16; weights stream in bf16. Heads are processed
in pairs (dim groups of 128) so projections, transposes, and PSUM tiles
use full 128-partition width. Intra-block attention runs in pass 1
(before the collective); the cross-chunk "inter" term is a cheap pass 2
after the AllGather, hidden behind the gate projection.
"""

import numpy as np
import ml_dtypes

from contextlib import ExitStack

import concourse.bacc as bacc
import concourse.mybir as mybir
import concourse.tile as tile
from concourse.bass_utils import run_bass_kernel_spmd
from concourse.masks import make_identity


AF = mybir.ActivationFunctionType
ALU = mybir.AluOpType
F32 = mybir.dt.float32
F32R = mybir.dt.float32r
BF16 = mybir.dt.bfloat16

H = 32
D = 64
BS = 256
HID = 2048
B = 2
S = 4096
NC = 8
T = S // 4            # tokens per core (1024)
NCH = T // 128        # 8 token chunks of 128
NBLK = T // BS        # 4 blocks per core
G = H // 2            # 16 head pairs (dim groups of 128)
GSPLIT = 12           # collective split point (groups 0:12 / 12:16)
KC = HID // 128       # 16 contraction chunks
LAYER_IDX = 0
NUM_LAYERS = 32
EPS = 1e-5

BF = ml_dtypes.bfloat16


def _decay():
    base = 1.0 / 2.0 ** (8.0 / H)
    rate = base ** (np.arange(H, dtype=np.float64) + 1.0)
    factor = 1.0 - LAYER_IDX / (NUM_LAYERS - 1 + 1e-5) + 1e-5
    slope = rate * factor                                  # (H,)
    r = np.arange(BS, dtype=np.float64) + 1.0
    qd = np.exp(-slope[:, None] * r[None, :])              # (H, BS) query decay
    kd = np.exp(-slope[:, None] * (BS - r[None, :]))       # (H, BS) key decay
    ij = r[:, None] - r[None, :]                           # i - j
    dd = np.where(
        ij[None] >= 0, np.exp(-slope[:, None, None] * ij[None]), 0.0
    )                                                      # (H, BS_i, BS_j)
    bd = np.exp(-slope * BS)                               # (H,) block decay
    return slope, qd, kd, dd, bd


def _build_nc():
    nc = bacc.Bacc(num_devices=NC)
    hsT = nc.declare_dram_parameter("hsT", [HID, T], BF16, isOutput=False)
    wvT = nc.declare_dram_parameter("wvT", [HID, H * D], BF16, isOutput=False)
    wqkT = nc.declare_dram_parameter("wqkT", [HID, G, 256], BF16, isOutput=False)
    gwT = nc.declare_dram_parameter("gwT", [HID, HID], BF16, isOutput=False)
    owT = nc.declare_dram_parameter("owT", [H * D, HID], BF16, isOutput=False)
    ddm = nc.declare_dram_parameter("ddm", [G, 128, 1024], BF16, isOutput=False)
    qdm = nc.declare_dram_parameter("qdm", [128, G, BS], BF16, isOutput=False)
    kdm = nc.declare_dram_parameter("kdm", [128, 2 * H], F32, isOutput=False)
    bdm = nc.declare_dram_parameter("bdm", [128, G], F32, isOutput=False)
    swm = nc.declare_dram_parameter("swm", [128, G * NC], F32, isOutput=False)
    nw = nc.declare_dram_parameter("nw", [128, 16], F32, isOutput=False)
    out = nc.declare_dram_parameter("out", [T, HID], F32, isOutput=True)

    # collective split in two group-halves so the first AllGather hides
    # under the second half of pass 1
    eloc_a = nc.dram_tensor("eloc_a", [2, GSPLIT, D, D], BF16)
    eloc_b = nc.dram_tensor("eloc_b", [2, G - GSPLIT, D, D], BF16)
    egath_a = nc.dram_tensor("egath_a", [NC, 2, GSPLIT, D, D], BF16, addr_space="Shared")
    egath_b = nc.dram_tensor("egath_b", [NC, 2, G - GSPLIT, D, D], BF16, addr_space="Shared")
    ssq_rt = nc.dram_tensor("ssq_rt", [T], F32)

    with tile.TileContext(nc, pool_alloc_mode="stack") as tc:
        # ---- constants -------------------------------------------------
        _c_ctx = ExitStack()
        c_pool = _c_ctx.enter_context(tc.tile_pool(name="c_pool", bufs=1))
        ident = c_pool.tile([128, 128], BF16, name="ident")
        make_identity(nc, ident[:])
        ones = c_pool.tile([128, 1], BF16, name="ones")
        nc.vector.memset(ones[:], 1.0)
        eps_sb = c_pool.tile([128, 1], F32, name="eps_sb")
        nc.vector.memset(eps_sb[:], EPS)
        kdm_sb = c_pool.tile([128, 2 * H], F32, name="kdm_sb")
        bdm_sb = c_pool.tile([128, G, 1], F32, name="bdm_sb")
        # swm_sb[p, cc, g, 0] = sw[2g + p//64, cc]
        swm_sb = c_pool.tile([128, NC, G, 1], F32, name="swm_sb")
        nw_sb = c_pool.tile([128, 16], F32, name="nw_sb")

        # ---- persistent activations ------------------------------------
        # xT DMAs go first in the sync queue so phase V starts ASAP;
        # small const loads follow.
        _xt_ctx = ExitStack()
        xt_pool = _xt_ctx.enter_context(tc.tile_pool(name="xt_pool", bufs=1))
        xT = xt_pool.tile([128, KC, T], BF16, name="xT")
        for q4 in range(4):
            nc.gpsimd.dma_start(
                xT[:, q4 * 4 : (q4 + 1) * 4, :],
                hsT[q4 * 512 : (q4 + 1) * 512, :].rearrange(
                    "(k p) t -> p k t", p=128
                ),
            )
        nc.sync.dma_start(kdm_sb[:], kdm[:])
        nc.sync.dma_start(bdm_sb[:, :, 0], bdm[:])
        nc.sync.dma_start(
            swm_sb[:, :, :, 0], swm.rearrange("p (c g) -> p c g", c=NC)
        )
        nc.sync.dma_start(nw_sb[:], nw[:])
        _q_ctx = ExitStack()
        q_pool = _q_ctx.enter_context(tc.tile_pool(name="q_pool", bufs=1))
        qT_sb = q_pool.tile([128, G, T], BF16, name="qT_sb")
        _ys_ctx = ExitStack()
        ys_pool = _ys_ctx.enter_context(tc.tile_pool(name="ys_pool", bufs=1))
        ys_sb = ys_pool.tile([128, G, T], BF16, name="ys_sb")
        _ce_ctx = ExitStack()
        ce_pool = _ce_ctx.enter_context(tc.tile_pool(name="ce_pool", bufs=1))
        c_sb = ce_pool.tile([128, G, NBLK, D], BF16, name="c_sb")
        E_sb = ce_pool.tile([128, G, D], BF16, name="E_sb")
        # weight pools sit below v_pool on the stack so their DMAs carry
        # no memory-reuse dependency on the previous phase's consumers
        _gw_ctx = ExitStack()
        gw_p = _gw_ctx.enter_context(tc.tile_pool(name="gw_p", bufs=3))
        # v_pool opened last among persistents: it is the only one released
        # mid-stream (stack allocator frees LIFO only)
        _v_ctx = ExitStack()
        v_pool = _v_ctx.enter_context(tc.tile_pool(name="v_pool", bufs=1))
        V_sb = v_pool.tile([128, NCH, H * D], BF16, name="V_sb")

        # ---- phase V: value projection (tok-major, all heads) -----------
        with tc.tile_pool(name="wv_p", bufs=4) as wv_p, tc.tile_pool(
            name="ps_v", bufs=1, space="PSUM"
        ) as ps_v:
            for n in range(4):
                pv = [
                    ps_v.tile([128, 512], F32, name=f"pv{m}") for m in range(NCH)
                ]
                for k2 in range(KC // 2):
                    wv_t = wv_p.tile([128, 2, 512], BF16, name="wv_t")
                    nc.sync.dma_start(
                        wv_t[:],
                        wvT[
                            k2 * 256 : (k2 + 1) * 256, n * 512 : (n + 1) * 512
                        ].rearrange("(ko p) c -> p ko c", p=128),
                    )
                    for kk in range(2):
                        k = 2 * k2 + kk
                        for m in range(NCH):
                            nc.tensor.matmul(
                                pv[m][:],
                                xT[:, k, m * 128 : (m + 1) * 128],
                                wv_t[:, kk, :],
                                start=(k == 0),
                                stop=(k == KC - 1),
                            )
                for m in range(NCH):
                    nc.scalar.activation(
                        V_sb[:, m, n * 512 : (n + 1) * 512], pv[m][:], AF.Silu
                    )

        # ---- pass 1: per head-pair: QK proj, C/E summary, intra attn ----
        with tc.tile_pool(name="wqk_p", bufs=3) as wqk_p, tc.tile_pool(
            name="dd_p", bufs=2
        ) as dd_p, tc.tile_pool(name="kt_p", bufs=2) as kt_p, tc.tile_pool(
            name="ktok_p", bufs=2
        ) as ktok_p, tc.tile_pool(name="vkd_p", bufs=2) as vkd_p, tc.tile_pool(
            name="awm_p", bufs=3
        ) as awm_p, tc.tile_pool(name="ps1", bufs=1, space="PSUM") as ps1:
            for g in range(G):
                wqk_a = wqk_p.tile([128, 8, 256], BF16, name="wqk_t")
                nc.sync.dma_start(
                    wqk_a[:],
                    wqkT[0:1024, g, :].rearrange("(ko p) c -> p ko c", p=128),
                )
                wqk_b = wqk_p.tile([128, 8, 256], BF16, name="wqk_t")
                nc.sync.dma_start(
                    wqk_b[:],
                    wqkT[1024:2048, g, :].rearrange("(ko p) c -> p ko c", p=128),
                )
                dd_g = dd_p.tile([128, 1024], BF16, name="dd_g")
                nc.sync.dma_start(dd_g[:], ddm[g])

                # q projection (dim-major, both heads: partitions 0:64/64:128)
                for half in range(2):
                    pq = ps1.tile([128, 512], F32, name="pq", tag="proj", bufs=2)
                    for k in range(KC):
                        wt = wqk_a if k < 8 else wqk_b
                        nc.tensor.matmul(
                            pq[:],
                            wt[:, k % 8, 0:128],
                            xT[:, k, half * 512 : (half + 1) * 512],
                            start=(k == 0),
                            stop=(k == KC - 1),
                        )
                    nc.scalar.activation(
                        qT_sb[:, g, half * 512 : (half + 1) * 512], pq[:], AF.Silu
                    )
                # k projection
                kT2 = kt_p.tile([128, T], BF16, name="kT2")
                for half in range(2):
                    pk = ps1.tile([128, 512], F32, name="pk", tag="proj", bufs=2)
                    for k in range(KC):
                        wt = wqk_a if k < 8 else wqk_b
                        nc.tensor.matmul(
                            pk[:],
                            wt[:, k % 8, 128:256],
                            xT[:, k, half * 512 : (half + 1) * 512],
                            start=(k == 0),
                            stop=(k == KC - 1),
                        )
                    nc.scalar.activation(
                        kT2[:, half * 512 : (half + 1) * 512], pk[:], AF.Silu
                    )

                # k -> tok-major via DMA (xbar) transpose: frees ~25us of
                # PE time; bufs=3 on the target pool for reuse distance
                k_tok = ktok_p.tile([128, NCH, 128], BF16, name="k_tok", bufs=3)
                for ch in range(NCH):
                    nc.sync.dma_start(
                        k_tok[:, ch, :],
                        kT2[:, ch * 128 : (ch + 1) * 128],
                        transpose=True,
                    )

                # v scaled by key-decay
                v_kd = vkd_p.tile([128, NCH, 128], BF16, name="v_kd")
                for m in range(NCH):
                    for hh in range(2):
                        h = 2 * g + hh
                        nc.vector.tensor_scalar_mul(
                            v_kd[:, m, hh * 64 : (hh + 1) * 64],
                            V_sb[:, m, g * 128 + hh * 64 : g * 128 + (hh + 1) * 64],
                            kdm_sb[:, 2 * h + (m % 2) : 2 * h + (m % 2) + 1],
                        )

                # block contributions C_jb = (k*kd)^T v (both heads at once)
                pc2 = ps1.tile([128, 4, 128], F32, name="pc2", tag="sm", bufs=3)
                for jb in range(NBLK):
                    for jc in range(2):
                        m = 2 * jb + jc
                        nc.tensor.matmul(
                            pc2[:, jb, :],
                            k_tok[:, m, :],
                            v_kd[:, m, :],
                            start=(jc == 0),
                            stop=(jc == 1),
                            skip_group_check=True,
                        )
                for hh in range(2):
                    sl = slice(hh * 64, (hh + 1) * 64)
                    nc.scalar.copy(
                        c_sb[sl, g, :, :], pc2[sl, :, hh * 64 : (hh + 1) * 64]
                    )
                # chunk summary E = sum_jb bd^(3-jb) C_jb
                nc.vector.tensor_copy(E_sb[:, g, :], c_sb[:, g, 0, :])
                for jb in range(1, NBLK):
                    nc.vector.scalar_tensor_tensor(
                        E_sb[:, g, :],
                        E_sb[:, g, :],
                        bdm_sb[:, g, :],
                        c_sb[:, g, jb, :],
                        ALU.mult,
                        ALU.add,
                    )

                # split collective: groups 0-11 gathered mid-pass-1 (latency
                # hides under remaining groups), 12-15 at the end; issued
                # right after the E chain so it fires before this group's
                # intra-attention work
                if g == GSPLIT - 1 or g == G - 1:
                    eh, gh, lo, ln = (
                        (eloc_a, egath_a, 0, GSPLIT)
                        if g < G - 1
                        else (eloc_b, egath_b, GSPLIT, G - GSPLIT)
                    )
                    nc.sync.dma_start(
                        eh[0].rearrange("g d e -> d g e"),
                        E_sb[0:64, lo : lo + ln, :],
                    )
                    nc.sync.dma_start(
                        eh[1].rearrange("g d e -> d g e"),
                        E_sb[64:128, lo : lo + ln, :],
                    )
                    nc.gpsimd.collective_compute(
                        "AllGather",
                        ALU.bypass,
                        replica_groups=[list(range(NC))],
                        ins=[eh[:]],
                        outs=[gh[:]],
                    )

                # intra-block attention
                for jb in range(NBLK):
                    awms = []
                    for hh in range(2):
                        hs = slice(hh * 64, (hh + 1) * 64)
                        paw = ps1.tile([128, 2, 256], F32, name="paw", tag="aw", bufs=2)
                        for jc in range(2):
                            nc.tensor.matmul(
                                paw[:, jc, :],
                                kT2[hs, jb * 256 + jc * 128 : jb * 256 + (jc + 1) * 128],
                                qT_sb[hs, g, jb * 256 : (jb + 1) * 256],
                                start=True,
                                stop=True,
                                skip_group_check=True,
                            )
                        awm = awm_p.tile([128, 2, 256], BF16, name="awm")
                        nc.vector.tensor_mul(
                            awm[:].rearrange("p a b -> p (a b)"),
                            paw[:].rearrange("p a b -> p (a b)"),
                            dd_g[:, hh * 512 : (hh + 1) * 512],
                        )
                        awms.append(awm)
                    pys = ps1.tile([128, 256], F32, name="pys", tag="sm", bufs=3)
                    for hh in range(2):
                        for jc in range(2):
                            m = 2 * jb + jc
                            nc.tensor.matmul(
                                pys[hh * 64 : (hh + 1) * 64, :],
                                V_sb[:, m, g * 128 + hh * 64 : g * 128 + (hh + 1) * 64],
                                awms[hh][:, jc, :],
                                start=(jc == 0),
                                stop=(jc == 1),
                                skip_group_check=True,
                            )
                    nc.scalar.copy(ys_sb[:, g, jb * 256 : (jb + 1) * 256], pys[:])

        _v_ctx.close()

        # ---- gate projection (overlaps the collective) ------------------
        _g_ctx = ExitStack()
        g_pool = _g_ctx.enter_context(tc.tile_pool(name="g_pool", bufs=1))
        gate_sb = g_pool.tile([128, G, T], BF16, name="gate_sb")
        _eg_ctx = ExitStack()
        eg_pool = _eg_ctx.enter_context(tc.tile_pool(name="eg_pool", bufs=1))
        EG = eg_pool.tile([128, G, NC, D], BF16, name="EG")
        qd_sb = eg_pool.tile([128, G, BS], BF16, name="qd_sb")

        def load_eg(hf):
            # gpsimd (SWDGE) queue: a pending wait on the collective must
            # not block the gate-weight stream on the sync queue
            gh, lo, ln = (
                (egath_a, 0, GSPLIT)
                if hf == 0
                else (egath_b, GSPLIT, G - GSPLIT)
            )
            for cc in range(NC):
                nc.gpsimd.dma_start(
                    EG[0:64, lo : lo + ln, cc, :],
                    gh[cc, 0].rearrange("g d e -> d g e"),
                )
                nc.gpsimd.dma_start(
                    EG[64:128, lo : lo + ln, cc, :],
                    gh[cc, 1].rearrange("g d e -> d g e"),
                )

        # ss4[:, jb, g, :] = chunk-start state for local block jb of pair g
        ss4 = g_pool.tile([128, NBLK, G, D], BF16, name="ss4")

        # ---- merged phase: gate proj + pass 2 (inter) + rmsnorm prep ----
        # pass-2 / prep work for group i-LAG is interleaved after gate
        # chunk i so the collective latency hides under the first LAG
        # gate chunks and the vector work overlaps gate matmuls.
        with tc.tile_pool(
            name="tmp_p", bufs=2
        ) as tmp_p, tc.tile_pool(name="sq_p2", bufs=2) as sq_p2, tc.tile_pool(
            name="ps_g", bufs=1, space="PSUM"
        ) as ps_g, tc.tile_pool(name="ps2", bufs=1, space="PSUM") as ps2, tc.tile_pool(
            name="ps_sq", bufs=1, space="PSUM"
        ) as ps_sq:
            # batched chunk-start state computation per group-half
            # (vector; each half waits only on its own AllGather)
            def ss4_init(hf):
                lo, ln = (0, GSPLIT) if hf == 0 else (GSPLIT, G - GSPLIT)
                gs = slice(lo, lo + ln)
                nc.vector.tensor_mul(
                    ss4[:, 0, gs, :],
                    EG[:, gs, 0, :],
                    swm_sb[:, 0, gs, :].broadcast_to([128, ln, D]),
                )
                for cc in range(1, NC):
                    tmp = tmp_p.tile([128, ln, D], BF16, name="tmp")
                    nc.vector.tensor_mul(
                        tmp[:],
                        EG[:, gs, cc, :],
                        swm_sb[:, cc, gs, :].broadcast_to([128, ln, D]),
                    )
                    nc.vector.tensor_add(ss4[:, 0, gs, :], ss4[:, 0, gs, :], tmp[:])
                for jb in range(1, NBLK):
                    tmp = tmp_p.tile([128, ln, D], BF16, name="tmp")
                    nc.vector.tensor_mul(
                        tmp[:],
                        ss4[:, jb - 1, gs, :],
                        bdm_sb[:, gs, :].broadcast_to([128, ln, D]),
                    )
                    nc.vector.tensor_add(
                        ss4[:, jb, gs, :], tmp[:], c_sb[:, gs, jb - 1, :]
                    )

            nc.gpsimd.dma_start(qd_sb[:], qdm[:])
            load_eg(0)
            load_eg(1)
            ss4_init(0)
            ssq0 = ps_sq.tile([1, 512], F32, name="ssq0")
            ssq1 = ps_sq.tile([1, 512], F32, name="ssq1")

            def pass2_group(g):
                pin = ps2.tile([128, NBLK, BS], F32, name="pin", bufs=2)
                for jb in range(NBLK):
                    for hh in range(2):
                        hs = slice(hh * 64, (hh + 1) * 64)
                        nc.tensor.matmul(
                            pin[hs, jb, :],
                            ss4[hs, jb, g, :],
                            qT_sb[hs, g, jb * 256 : (jb + 1) * 256],
                            start=True,
                            stop=True,
                            skip_group_check=True,
                        )
                tmp3 = tmp_p.tile([128, NBLK, BS], BF16, name="tmp3")
                nc.vector.tensor_mul(
                    tmp3[:],
                    pin[:],
                    qd_sb[:, g : g + 1, :].broadcast_to([128, NBLK, BS]),
                )
                nc.vector.tensor_add(
                    ys_sb[:, g, :],
                    ys_sb[:, g, :],
                    tmp3[:].rearrange("p a b -> p (a b)"),
                )
                # rmsnorm prep for this (now final) chunk of ys
                sq = sq_p2.tile([128, T], BF16, name="sq")
                nc.scalar.activation(sq[:], ys_sb[:, g, :], AF.Square)
                for half in range(2):
                    nc.tensor.matmul(
                        [ssq0, ssq1][half][:],
                        ones[:],
                        sq[:, half * 512 : (half + 1) * 512],
                        start=(g == 0),
                        stop=(g == G - 1),
                        skip_group_check=True,
                    )
                nc.vector.scalar_tensor_tensor(
                    gate_sb[:, g, :],
                    ys_sb[:, g, :],
                    nw_sb[:, g : g + 1],
                    gate_sb[:, g, :],
                    ALU.mult,
                    ALU.mult,
                )  # gate <- ys * norm_w * gate (per dim-chunk g)

            # groups 0..GSPLIT interleave with gate chunks at lag 1; the
            # last 4 groups run after all gate chunks so collective B has
            # the whole gate phase to complete without stalling the PE FIFO
            for i in range(16 + (G - GSPLIT)):
                if i < 16:
                    gm = i
                    gw_t = gw_p.tile([128, KC, 128], BF16, name="gw_t")
                    nc.sync.dma_start(
                        gw_t[:],
                        gwT[:, gm * 128 : (gm + 1) * 128].rearrange(
                            "(ko p) c -> p ko c", p=128
                        ),
                    )
                    for gn in range(2):
                        pg = ps_g.tile([128, 512], F32, name="pg", bufs=2)
                        for k in range(KC):
                            nc.tensor.matmul(
                                pg[:],
                                gw_t[:, k, :],
                                xT[:, k, gn * 512 : (gn + 1) * 512],
                                start=(k == 0),
                                stop=(k == KC - 1),
                            )
                        nc.scalar.activation(
                            gate_sb[:, gm, gn * 512 : (gn + 1) * 512],
                            pg[:],
                            AF.Sigmoid,
                        )
                if i == 16:
                    ss4_init(1)
                if 1 <= i <= GSPLIT:
                    pass2_group(i - 1)
                elif i > 15:
                    pass2_group(i - 4)

            ssq_sb = sq_p2.tile([1, T], F32, name="ssq_sb")
            nc.vector.tensor_copy(ssq_sb[:, 0:512], ssq0[:])
            nc.vector.tensor_copy(ssq_sb[:, 512:1024], ssq1[:])
            nc.sync.dma_start(ssq_rt[:], ssq_sb[:])
        _eg_ctx.close()

        # ---- phase F: output projection --------------------------------
        with tc.tile_pool(name="sq_p", bufs=2) as sq_p:
            ns_l = sq_p.tile([128, NCH], F32, name="ns_l")
            nc.sync.dma_start(ns_l[:], ssq_rt.rearrange("(c p) -> p c", p=128))
            ns_t = sq_p.tile([128, NCH], F32, name="ns_t")
            nc.scalar.activation(
                ns_t[:], ns_l[:], AF.Sqrt, bias=eps_sb[:, 0:1], scale=1.0 / (H * D)
            )
            ns_sb = sq_p.tile([128, NCH], F32, name="ns_sb")
            nc.vector.reciprocal(ns_sb[:], ns_t[:])

            with tc.tile_pool(name="ow_p", bufs=4) as ow_p, tc.tile_pool(
                name="oo_p", bufs=1
            ) as oo_p, tc.tile_pool(name="ps_o", bufs=1, space="PSUM") as ps_o:
                for n in range(4):
                    po = [
                        ps_o.tile([128, 512], F32, name=f"po{m}") for m in range(NCH)
                    ]
                    for k2 in range(KC // 2):
                        ow_t = ow_p.tile([128, 2, 512], BF16, name="ow_t")
                        nc.sync.dma_start(
                            ow_t[:],
                            owT[
                                k2 * 256 : (k2 + 1) * 256, n * 512 : (n + 1) * 512
                            ].rearrange("(ko p) c -> p ko c", p=128),
                        )
                        for kk in range(2):
                            k = 2 * k2 + kk
                            for m in range(NCH):
                                nc.tensor.matmul(
                                    po[m][:],
                                    gate_sb[:, k, m * 128 : (m + 1) * 128],
                                    ow_t[:, kk, :],
                                    start=(k == 0),
                                    stop=(k == KC - 1),
                                )
                    oo_all = oo_p.tile([128, NCH, 512], F32, name="oo_all")
                    for mh in range(4):
                        for mm in range(2):
                            m = 2 * mh + mm
                            nc.vector.tensor_scalar_mul(
                                oo_all[:, m, :], po[m][:], ns_sb[:, m : m + 1]
                            )
                        nc.sync.dma_start(
                            out[
                                mh * 256 : (mh + 1) * 256,
                                n * 512 : (n + 1) * 512,
                            ].rearrange("(m p) c -> p m c", p=128),
                            oo_all[:, mh * 2 : (mh + 1) * 2, :],
                        )
        _g_ctx.close()
        _gw_ctx.close()
        _ce_ctx.close()
        _ys_ctx.close()
        _q_ctx.close()
        _xt_ctx.close()
        _c_ctx.close()
    nc.finalize()
    return nc


_CACHE = {}


def _get_nc():
    if "nc" not in _CACHE:
        _CACHE["nc"] = _build_nc()
    return _CACHE["nc"]


def _host_prep(hidden_states, qkv_w, out_w, gate_w, norm_w):
    slope, qd, kd, dd, bd = _decay()
    w3 = qkv_w.reshape(H, 3 * D, HID)
    wq = w3[:, 0:D, :].reshape(H * D, HID)
    wk = w3[:, D : 2 * D, :].reshape(H * D, HID)
    wv = w3[:, 2 * D : 3 * D, :].reshape(H * D, HID)
    # wqkT[:, g, 0:128] = q dims of heads 2g,2g+1; [:, g, 128:256] = k dims
    wqkT = np.concatenate(
        [
            np.ascontiguousarray(wq.T).reshape(HID, G, 128),
            np.ascontiguousarray(wk.T).reshape(HID, G, 128),
        ],
        axis=2,
    ).astype(BF)
    wvT = np.ascontiguousarray(wv.T).astype(BF)
    gwT = np.ascontiguousarray(gate_w.T).astype(BF)
    owT = np.ascontiguousarray(out_w.T).astype(BF)

    # ddm[g, p, h*512 + jc*256 + i] = dd[2g+h, i, jc*128+p]
    dd_t = dd.transpose(0, 2, 1)  # (H, j, i)
    ddm = np.ascontiguousarray(
        dd_t.reshape(G, 2, 2, 128, BS).transpose(0, 3, 1, 2, 4).reshape(G, 128, 1024)
    ).astype(BF)
    # qdm[p, g, i] = qd[2g + p//64, i]
    qdm = np.ascontiguousarray(
        np.broadcast_to(qd.reshape(G, 2, 1, BS), (G, 2, 64, BS))
        .transpose(1, 2, 0, 3)
        .reshape(128, G, BS)
    ).astype(BF)
    # kdm[p, 2h+parity] = kd[h, parity*128+p]
    kdm = np.ascontiguousarray(
        kd.reshape(H, 2, 128).transpose(2, 0, 1).reshape(128, 2 * H)
    ).astype(np.float32)
    # bdm[p, g] = bd[2g + p//64]
    bdm = np.ascontiguousarray(
        np.broadcast_to(bd.reshape(G, 2, 1), (G, 2, 64)).transpose(1, 2, 0).reshape(128, G)
    ).astype(np.float32)
    nw = np.ascontiguousarray(norm_w.reshape(16, 128).T).astype(np.float32)

    shared = dict(wqkT=wqkT, wvT=wvT, gwT=gwT, owT=owT, ddm=ddm, qdm=qdm,
                  kdm=kdm, bdm=bdm, nw=nw)
    in_maps = []
    for c in range(NC):
        bb, p = c // 4, c % 4
        hsT = np.ascontiguousarray(
            hidden_states[bb, p * T : (p + 1) * T, :].T
        ).astype(BF)
        sw = np.zeros((H, NC), dtype=np.float64)
        for cc in range(NC):
            if cc // 4 == bb and cc % 4 < p:
                sw[:, cc] = bd ** (4.0 * (p - 1 - (cc % 4)))
        # swm[p_, cc*G+g] = sw[2g + p_//64, cc]  (cc-major)
        swm = np.ascontiguousarray(
            np.broadcast_to(sw.reshape(G, 2, 1, NC), (G, 2, 64, NC))
            .transpose(1, 2, 3, 0)
            .reshape(128, NC * G)
        ).astype(np.float32)
        in_maps.append(dict(hsT=hsT, swm=swm, **shared))
    return in_maps


def _run(inputs, trace=False):
    nc = _get_nc()
    in_maps = _host_prep(
        np.asarray(inputs["hidden_states"], dtype=np.float32),
        np.asarray(inputs["qkv_w"], dtype=np.float32),
        np.asarray(inputs["out_w"], dtype=np.float32),
        np.asarray(inputs["gate_w"], dtype=np.float32),
        np.asarray(inputs["norm_w"], dtype=np.float32),
    )
    res = run_bass_kernel_spmd(nc, in_maps, core_ids=list(range(NC)), trace=trace)
    full = np.empty((B, S, HID), dtype=np.float32)
    for c in range(NC):
        bb, p = c // 4, c % 4
        full[bb, p * T : (p + 1) * T, :] = res.results[c]["out"]
    return full, res


def kernel(**inputs):
    return _run(inputs, trace=False)[0]


def kernel_traced(**inputs):
    full, res = _run(inputs, trace=True)
    return full, res.exec_time_ns
